# revision 8
# baseline (speedup 1.0000x reference)
"""GCC-PHAT spatial likelihood grid kernel for Trainium2 (8 NeuronCores).

Self-contained: kernel(**inputs) -> np.ndarray. Shards the batch over the 8
cores (pure data parallel), runs a Bass/Tile kernel per core, gathers.
"""

import json

import numpy as np
from contextlib import ExitStack

import concourse.bass as bass
import concourse.bacc as bacc
import concourse.mybir as mybir
from concourse import tile, library_config
from concourse.bass_utils import run_bass_kernel_spmd
from concourse.tile import TileContext
from bass_rust import ScopedClock

# ---------------------------------------------------------------------------
# Workaround 1: this walrus build allows at most one semaphore wait per
# instruction. Post-process the BIR JSON: excess waits move onto NoOps
# inserted just before the offending instruction (same engine, so ordering
# is preserved).
_uid = [0]


def _fix_module(m):
    for f in m.get("functions", []):
        for bb in f.get("blocks", []):
            insts = bb.get("instructions")
            if not insts:
                continue
            out = []
            changed = False
            for ins in insts:
                si = ins.get("sync_info")
                ow = (si or {}).get("on_wait") or []
                if len(ow) > 1:
                    changed = True
                    for w in ow[1:]:
                        _uid[0] += 1
                        out.append({
                            "engine": ins["engine"], "ins": [], "outs": [],
                            "name": f"WFix-{_uid[0]}", "opcode": "NoOp",
                            "sync_info": {"on_update": [], "on_wait": [w]},
                        })
                    si["on_wait"] = ow[:1]
                out.append(ins)
            if changed:
                bb["instructions"] = out
    return m


_orig_to_json_bytes = bass.Bass.to_json_bytes


def _to_json_bytes(self):
    return json.dumps(_fix_module(json.loads(_orig_to_json_bytes(self)))).encode()


bass.Bass.to_json_bytes = _to_json_bytes

# Workaround 2: the TileContext tail Drain gets zero wait slots here; move
# the end-of-kernel waits onto SP NoOps emitted before the drain.


def _drain_and_barrier(self, tick_clock, wait_clock):
    nc = self.nc
    first_nop = nc.sync.nop()
    wait_clock.add_sem_waits(first_nop.ins, ScopedClock({None: tick_clock.global_clock}))
    si = first_nop.ins.sync_info
    if si is not None and len(si.on_wait) > 1:
        waits = list(si.on_wait)
        first_nop.ins.sync_info = mybir.SyncInfo(
            on_wait=waits[:1], on_update=list(si.on_update)
        )
        for w in waits[1:]:
            nop = nc.sync.nop()
            nop.ins.sync_info = mybir.SyncInfo(on_wait=[w], on_update=[])
    nc.sync.drain()
    nc.all_engine_barrier()
    popped = nc._tile_sem_poison_stack.pop()
    assert popped is self._sem_poison
    nc.clear_and_free_semaphores(list(self.sems.allocated().values()))
    nc.all_engine_barrier()


TileContext._drain_and_barrier = _drain_and_barrier

# ---------------------------------------------------------------------------
FP = mybir.dt.float32
I32 = mybir.dt.int32
I16 = mybir.dt.int16
ALU = mybir.AluOpType
ACTF = mybir.ActivationFunctionType

SR = 16000.0
SPEED = 343.0
G = 128
B, K, M, N = 16, 4, 4, 8192
P1, P2 = 128, 64  # N = P1*P2 ; n = p*64 + j ; bin k = k2*128 + k1
PAIRS = [(0, 1), (0, 2), (0, 3), (1, 2), (1, 3), (2, 3)]
NPA = 6
NB = 2
NBK = NB * K  # 8
NS = NBK * M  # 32
NPAIR = NBK * NPA  # 48
T1LO, T1HI = 56, 72
NT1 = T1HI - T1LO  # 16
NTAB = NT1 * P2  # 1024
NQ = G * G  # 16384
G2HI = G // 16  # 8


def make_consts():
    c = {}
    k1 = np.arange(P1)
    j = np.arange(P2)
    k2 = np.arange(P2)
    t2 = np.arange(P2)
    p = np.arange(P1)

    ang = 2 * np.pi * np.outer(p, k1) / P1
    c["a_re"] = np.cos(ang).astype(np.float32)  # [p, k1]
    c["a_im"] = (-np.sin(ang)).astype(np.float32)

    angt = 2 * np.pi * np.outer(k1, j) / N  # fwd twiddle e^{-i...}
    c["twf_re"] = np.tile(np.cos(angt).astype(np.float32), (1, NS))  # [128, 2048]
    c["twf_im"] = np.tile((-np.sin(angt)).astype(np.float32), (1, NS))

    angb = 2 * np.pi * np.outer(j, k2) / P2  # W64 = e^{-i...}
    c["b_re"] = np.cos(angb).astype(np.float32)  # [j, k2]
    c["b_im"] = (-np.sin(angb)).astype(np.float32)
    c["b_im_neg"] = -c["b_im"]

    kk = k1[:, None] + P1 * k2[None, :]  # [k1, k2] bin index
    o = np.arange(-5, 5)
    D = np.exp(2j * np.pi * kk[..., None] * o / N).sum(-1)
    DW = D * ((-1.0) ** kk) / N
    c["dw_re"] = np.tile(np.ascontiguousarray(DW.real).astype(np.float32), (1, NS))
    c["dw_im"] = np.tile(np.ascontiguousarray(DW.imag).astype(np.float32), (1, NS))

    angi = 2 * np.pi * np.outer(k2, t2) / P2  # E64 = e^{+i...}
    c["e64_re"] = np.cos(angi).astype(np.float32)  # [k2, t2]
    c["e64_im"] = np.sin(angi).astype(np.float32)
    c["e64_im_neg"] = -c["e64_im"]

    angti = 2 * np.pi * np.outer(k1, t2) / N  # inv twiddle e^{+i...}
    c["twi_re"] = np.tile(np.cos(angti).astype(np.float32), (1, NPAIR))  # [128, 3072]
    c["twi_im"] = np.tile(np.sin(angti).astype(np.float32), (1, NPAIR))

    t1 = np.arange(T1LO, T1HI)
    ango = 2 * np.pi * np.outer(k1, t1) / P1  # E128 = e^{+i...}
    c["e128_re"] = np.cos(ango).astype(np.float32)  # [k1, 16]
    c["e128_im_neg"] = (-np.sin(ango)).astype(np.float32)

    c["ident"] = np.eye(128, dtype=np.float32)

    # group-broadcast selector: lhsT [8, 128]; out row p <- in row p//16
    r16 = np.zeros((NBK, 128), np.float32)
    for g in range(NBK):
        r16[g, g * 16 : (g + 1) * 16] = 1.0
    c["repl16"] = r16

    # grid parameter fields (wrapped layout): partition = (bk, g2lo), free = (g1, g2hi)
    t_lin = np.linspace(0.0, 1.0, G).astype(np.float32)
    g2lo = (p % 16)[:, None, None]
    g1 = np.arange(G)[None, :, None]
    g2hi = np.arange(G2HI)[None, None, :]
    gxc = np.broadcast_to(t_lin[g1], (128, G, G2HI))
    gyc = t_lin[(g2hi * 16 + g2lo)] * np.ones((128, G, G2HI), np.float32)
    c["gxc"] = np.ascontiguousarray(gxc.reshape(128, NQ // 16), np.float32)
    c["gyc"] = np.ascontiguousarray(gyc.reshape(128, NQ // 16), np.float32)
    return c


CONST_SPECS = [
    ("a_re", [128, 128]), ("a_im", [128, 128]),
    ("twf_re", [128, 64 * NS]), ("twf_im", [128, 64 * NS]),
    ("b_re", [64, 64]), ("b_im", [64, 64]), ("b_im_neg", [64, 64]),
    ("dw_re", [128, 64 * NS]), ("dw_im", [128, 64 * NS]),
    ("e64_re", [64, 64]), ("e64_im", [64, 64]), ("e64_im_neg", [64, 64]),
    ("twi_re", [128, 64 * NPAIR]), ("twi_im", [128, 64 * NPAIR]),
    ("e128_re", [128, NT1]), ("e128_im_neg", [128, NT1]),
    ("ident", [128, 128]),
    ("repl16", [NBK, 128]),
    ("gxc", [128, NQ // 16]), ("gyc", [128, NQ // 16]),
]


def shard_core(signal, mic, room, core):
    b0 = core * NB
    sig = np.ascontiguousarray(signal[b0 : b0 + NB].transpose(2, 0, 1, 3).reshape(NS, N))
    micc = mic[b0 : b0 + NB]
    roomc = room[b0 : b0 + NB]
    pp = np.arange(128)
    bk = pp // 16
    b = bk // K
    k = bk % K
    aux = np.zeros((128, 12), np.float32)
    aux[:, 0] = roomc[b, 0]
    aux[:, 1] = roomc[b, 1]
    for m in range(M):
        aux[:, 2 + m] = micc[b, k, m, 0]
        aux[:, 6 + m] = micc[b, k, m, 1]
    return {"signal": sig, "aux": aux}


def build_kernel():
    nc = bacc.Bacc("TRN2", target_bir_lowering=False, debug=False)
    sig_d = nc.declare_dram_parameter("signal", [NS, N], FP, isOutput=False)
    aux_d = nc.declare_dram_parameter("aux", [128, 12], FP, isOutput=False)
    cd = {
        name: nc.declare_dram_parameter(name, shape, FP, isOutput=False)
        for name, shape in CONST_SPECS
    }
    out_d = nc.declare_dram_parameter("out", [NBK, NQ], FP, isOutput=True)

    with tile.TileContext(nc) as tc:
        with ExitStack() as top:
            # small constants that live for the whole kernel
            cpool = top.enter_context(tc.tile_pool(name="consts", bufs=1))
            SMALL = {"a_re", "a_im", "b_re", "b_im", "b_im_neg", "e64_re", "e64_im",
                     "e64_im_neg", "e128_re", "e128_im_neg", "ident", "repl16"}
            ct = {}
            for name, shape in CONST_SPECS:
                if name in SMALL:
                    t = cpool.tile(shape, FP, tag=name, name=name)
                    nc.sync.dma_start(t[:], cd[name][:, :])
                    ct[name] = t
            aux = cpool.tile([128, 12], FP, tag="aux", name="aux")
            nc.sync.dma_start(aux[:], aux_d[:, :])
            bias_round = cpool.tile([128, 1], FP, tag="bias_round", name="bias_round")
            nc.vector.memset(bias_round[:], 512.0)
            bias_eps = cpool.tile([128, 1], FP, tag="bias_eps", name="bias_eps")
            nc.vector.memset(bias_eps[:], 1e-18)

            # gather inputs persist until the gather phase
            gpool = top.enter_context(tc.tile_pool(name="gin", bufs=1))
            gdata = [gpool.tile([128, NTAB], FP, tag=f"gdata{i}", name=f"gdata{i}") for i in range(NPA)]
            gidx = [gpool.tile([128, NTAB], I16, tag=f"gidx{i}", name=f"gidx{i}") for i in range(NPA)]

            # ---------------- geometry / index pipeline ----------------
            with ExitStack() as geo:
                gp = geo.enter_context(tc.tile_pool(name="geo", bufs=1))
                NF = NQ // 16  # 1024
                gxc = gp.tile([128, NF], FP, tag="gxc", name="gxc")
                nc.sync.dma_start(gxc[:], cd["gxc"][:, :])
                gyc = gp.tile([128, NF], FP, tag="gyc", name="gyc")
                nc.sync.dma_start(gyc[:], cd["gyc"][:, :])
                gx = gp.tile([128, NF], FP, tag="gx", name="gx")
                gy = gp.tile([128, NF], FP, tag="gy", name="gy")
                nc.vector.tensor_scalar_mul(gx[:], gxc[:], aux[:, 0:1])
                nc.vector.tensor_scalar_mul(gy[:], gyc[:], aux[:, 1:2])
                dist = [gp.tile([128, NF], FP, tag=f"dist{m}", name=f"dist{m}") for m in range(M)]
                negm = gp.tile([128, 8], FP, tag="negm", name="negm")
                nc.scalar.activation(negm[:], aux[:, 2:10], ACTF.Copy, scale=-1.0)
                for m in range(M):
                    d2 = gp.tile([128, NF], FP, tag="d2", name="d2")
                    dy2 = gp.tile([128, NF], FP, tag="dy2", name="dy2")
                    nc.scalar.activation(d2[:], gx[:], ACTF.Square, bias=negm[:, m : m + 1])
                    nc.scalar.activation(
                        dy2[:], gy[:], ACTF.Square, bias=negm[:, 4 + m : 5 + m]
                    )
                    nc.vector.tensor_add(d2[:], d2[:], dy2[:])
                    nc.scalar.activation(dist[m][:], d2[:], ACTF.Sqrt)
                for i, (mi, mj) in enumerate(PAIRS):
                    ds = gp.tile([128, NF], FP, tag="ds", name="ds")
                    dsi = gp.tile([128, NF], I32, tag="dsi", name="dsi")
                    d16 = gp.tile([128, NF], I32, tag="d16", name="d16")
                    dl = gp.tile([128, NF], I32, tag="dl", name="dl")
                    nc.vector.tensor_sub(ds[:], dist[mi][:], dist[mj][:])
                    # dlocal+0.5 = ds*(SR/343) + 512.5 ; then clamp, truncate
                    nc.scalar.activation(
                        ds[:], ds[:], ACTF.Identity,
                        bias=bias_round[:, 0:1], scale=float(np.float32(SR) / np.float32(SPEED)),
                    )
                    # no clamp: |dist_i - dist_j| < sqrt(72) m in a <=6m room,
                    # so the index stays within [116, 908] of the 1024 window
                    nc.vector.tensor_copy(dsi[:], ds[:])
                    # permuted index d' = (d & 63)*16 + (d >> 6)
                    nc.vector.tensor_scalar(dl[:], dsi[:], 63, None, op0=ALU.bitwise_and)
                    nc.vector.tensor_scalar(
                        d16[:], dsi[:], 6, None, op0=ALU.logical_shift_right
                    )
                    nc.vector.scalar_tensor_tensor(
                        gidx[i][:], dl[:], 16, d16[:], op0=ALU.mult, op1=ALU.add
                    )

            # ------------- forward FFT + spectra + inverse (half/pair split) ----
            # Signal order is now s = m*NBK + bk (mics major), so half h of the
            # signal range covers mics {2h, 2h+1}. Front tiles are split per
            # half and the post-PHAT stages per pair so that pair 0's table
            # (and its gather) is ready long before the full front finishes.
            class MPool:
                def __init__(self, name, space="SBUF", bufs=1, side=None):
                    self.cm = tc.tile_pool(name=name, bufs=bufs, space=space, side=side)
                    self.pool = self.cm.__enter__()
                def tile(self, *a, **kw):
                    return self.pool.tile(*a, **kw)
                def close(self):
                    self.cm.__exit__(None, None, None)

            NSH = NS // 2  # 16 signals per half
            HC = NSH * P2  # 1024 cols per half

            # persistent-front pools (tiles span both halves; h-major order
            # below puts mic-{0,1} FFT + pair 0's whole chain + gather 0
            # ahead of the mic-{2,3} FFT in every engine queue)
            p_go = MPool("p_go", side="left", bufs=1)
            acc = p_go.tile([128, NQ], FP, tag="acc", name="acc")
            m128a = p_go.tile([128, 1], FP, tag="m128a", name="m128a")
            p_q = MPool("p_q", side="left", bufs=2)
            p_gb = MPool("p_gb", side="left", bufs=1)
            p_ytw = MPool("p_ytw", side="left")
            p_a = MPool("p_a", side="left")
            psA = MPool("psA", space="PSUM", bufs=1)
            twf_re = p_a.tile([128, P2], FP, name="twf_re")
            nc.sync.dma_start(twf_re[:], cd["twf_re"][:, 0:P2])
            twf_im = p_a.tile([128, P2], FP, name="twf_im")
            nc.sync.dma_start(twf_im[:], cd["twf_im"][:, 0:P2])
            twfr_b = twf_re[:].rearrange("p (o j) -> p o j", o=1).broadcast_to((128, 8, P2))
            twfi_b = twf_im[:].rearrange("p (o j) -> p o j", o=1).broadcast_to((128, 8, P2))
            p_yt = MPool("p_yt", side="right")

            p_z = MPool("p_z", side="left")
            p_uv = MPool("p_uv", side="right")
            u_mre, u_mim, v_mre, v_mim = {}, {}, {}, {}
            p_n = MPool("p_n", side="right", bufs=1)
            dw_res = p_n.tile([128, P2], FP, tag="dwr", name="dw_res")
            nc.sync.dma_start(dw_res[:], cd["dw_re"][:, 0:P2])
            dw_ims = p_n.tile([128, P2], FP, tag="dwi", name="dw_ims")
            nc.sync.dma_start(dw_ims[:], cd["dw_im"][:, 0:P2])
            dw_re3 = dw_res[:].rearrange("p (o j) -> p o j", o=1).broadcast_to((128, NSH, P2))
            dw_im3 = dw_ims[:].rearrange("p (o j) -> p o j", o=1).broadcast_to((128, NSH, P2))
            p_bk = MPool("p_bk", side="right", bufs=1)
            twi_res = p_bk.tile([128, P2], FP, tag="twir", name="twi_res")
            nc.sync.dma_start(twi_res[:], cd["twi_re"][:, 0:P2])
            twi_ims = p_bk.tile([128, P2], FP, tag="twii", name="twi_ims")
            nc.sync.dma_start(twi_ims[:], cd["twi_im"][:, 0:P2])
            twir_b = twi_res[:].rearrange("p (o t) -> p o t", o=1).broadcast_to((128, 8, P2))
            twii_b = twi_ims[:].rearrange("p (o t) -> p o t", o=1).broadcast_to((128, 8, P2))
            NQH = NQ // 2
            NQQ = NQ // 4

            def mview(tiles, m):
                return tiles[m][:].rearrange("p (bk k) -> p bk k", bk=NBK)

            for h in range(2):

                # step A + twiddle for this half
                x2h = p_a.tile([128, NSH, P2], FP, tag="x2", name=f"x2{h}")
                nc.sync.dma_start(
                    x2h[:],
                    sig_d[h * NSH : (h + 1) * NSH, :].rearrange("s (p j) -> p s j", p=128),
                )
                x2f = x2h[:].rearrange("p s j -> p (s j)")
                z_lc_re, z_lc_im = {}, {}
                for lc in range(2):
                    sl = slice(lc * 512, (lc + 1) * 512)
                    yre = psA.tile([128, 512], FP, tag="t1", name="yre")
                    yim = psA.tile([128, 512], FP, tag="t2", name="yim")
                    nc.tensor.matmul(yre[:], ct["a_re"][:], x2f[:, sl])
                    nc.tensor.matmul(yim[:], ct["a_im"][:], x2f[:, sl])
                    tmp = p_a.tile([128, 512], FP, tag="twtmp", name="twtmp")
                    ytwr = p_ytw.tile([128, 512], FP, tag="ytw_re", name=f"ytw_re{h}{lc}")
                    ytwi = p_ytw.tile([128, 512], FP, tag="ytw_im", name=f"ytw_im{h}{lc}")
                    yre3 = yre[:].rearrange("p (s j) -> p s j", j=P2)
                    yim3 = yim[:].rearrange("p (s j) -> p s j", j=P2)
                    tmp3 = tmp[:].rearrange("p (s j) -> p s j", j=P2)
                    ytw_re3 = ytwr[:].rearrange("p (s j) -> p s j", j=P2)
                    ytw_im3 = ytwi[:].rearrange("p (s j) -> p s j", j=P2)
                    nc.vector.tensor_mul(tmp3, yre3, twfr_b)
                    nc.vector.tensor_mul(ytw_re3, yim3, twfi_b)
                    nc.vector.tensor_sub(ytw_re3, tmp3, ytw_re3)
                    nc.vector.tensor_mul(tmp3, yre3, twfi_b)
                    nc.vector.tensor_mul(ytw_im3, yim3, twfr_b)
                    nc.vector.tensor_add(ytw_im3, tmp3, ytw_im3)
                    # transpose + step B, one 4-signal chunk at a time
                    zre = psA.tile([128, 512], FP, tag="t1", name="zre")
                    zim = psA.tile([128, 512], FP, tag="t2", name="zim")
                    for c4 in range(2):
                        yTr = p_yt.tile([64, 512], FP, tag="ytTc_re", name=f"yTr{h}{lc}{c4}")
                        yTi = p_yt.tile([64, 512], FP, tag="ytTc_im", name=f"yTi{h}{lc}{c4}")
                        for srcv, dst in [(ytwr, yTr), (ytwi, yTi)]:
                            pt = psA.tile([64, 512], FP, tag="t3", name="ptr")
                            for k in range(4):
                                ls = c4 * 4 + k
                                nc.tensor.transpose(
                                    pt[:, k * 128 : (k + 1) * 128],
                                    srcv[:, ls * 64 : (ls + 1) * 64], ct["ident"][:],
                                )
                            nc.scalar.copy(dst[:], pt[:])
                        for col in range(4):
                            si = c4 * 4 + col
                            lre = yTr[:, col * 128 : (col + 1) * 128]
                            lim = yTi[:, col * 128 : (col + 1) * 128]
                            osl = slice(si * 64, (si + 1) * 64)
                            nc.tensor.matmul(zre[:, osl], lre, ct["b_re"][:], start=True, stop=False)
                            nc.tensor.matmul(zre[:, osl], lim, ct["b_im_neg"][:], start=False, stop=True)
                            nc.tensor.matmul(zim[:, osl], lre, ct["b_im"][:], start=True, stop=False)
                            nc.tensor.matmul(zim[:, osl], lim, ct["b_re"][:], start=False, stop=True)
                    z_lc_re[lc] = p_z.tile([128, 512], FP, tag="z_re", name=f"z_re{h}{lc}")
                    z_lc_im[lc] = p_z.tile([128, 512], FP, tag="z_im", name=f"z_im{h}{lc}")
                    nc.scalar.copy(z_lc_re[lc][:], zre[:])
                    nc.scalar.copy(z_lc_im[lc][:], zim[:])
                # PHAT + Dirichlet per mic (chunk ph <-> mic 2h+ph); V_3 unused
                for ph in range(2):
                    m = 2 * h + ph
                    nrm = p_n.tile([128, 512], FP, tag="nrm", name="nrm")
                    tmp2 = p_n.tile([128, 512], FP, tag="tmp2", name="tmp2")
                    nc.scalar.activation(nrm[:], z_lc_re[ph][:], ACTF.Square)
                    nc.scalar.activation(tmp2[:], z_lc_im[ph][:], ACTF.Square)
                    nc.vector.tensor_add(nrm[:], nrm[:], tmp2[:])
                    nc.scalar.activation(nrm[:], nrm[:], ACTF.Sqrt, bias=bias_eps[:, 0:1])
                    nc.vector.reciprocal(nrm[:], nrm[:])
                    u_mre[m] = p_uv.tile([128, 512], FP, tag=f"um_re{m % 2}", name=f"u_re_m{m}")
                    u_mim[m] = p_uv.tile([128, 512], FP, tag=f"um_im{m % 2}", name=f"u_im_m{m}")
                    nc.vector.tensor_mul(u_mre[m][:], z_lc_re[ph][:], nrm[:])
                    nc.vector.tensor_mul(u_mim[m][:], z_lc_im[ph][:], nrm[:])
                    if m < 3:
                        v_mre[m] = p_uv.tile([128, 512], FP, tag=f"vm_re{m}", name=f"v_re_m{m}")
                        v_mim[m] = p_uv.tile([128, 512], FP, tag=f"vm_im{m}", name=f"v_im_m{m}")
                        ur3 = u_mre[m][:].rearrange("p (s j) -> p s j", j=P2)
                        ui3 = u_mim[m][:].rearrange("p (s j) -> p s j", j=P2)
                        vr3 = v_mre[m][:].rearrange("p (s j) -> p s j", j=P2)
                        vi3 = v_mim[m][:].rearrange("p (s j) -> p s j", j=P2)
                        tm3 = tmp2[:].rearrange("p (s j) -> p s j", j=P2)
                        dw_re8 = dw_res[:].rearrange("p (o j) -> p o j", o=1).broadcast_to((128, 8, P2))
                        dw_im8 = dw_ims[:].rearrange("p (o j) -> p o j", o=1).broadcast_to((128, 8, P2))
                        nc.vector.tensor_mul(vr3, ur3, dw_re8)
                        nc.vector.tensor_mul(tm3, ui3, dw_im8)
                        nc.vector.tensor_sub(vr3, vr3, tm3)
                        nc.vector.tensor_mul(vi3, ur3, dw_im8)
                        nc.vector.tensor_mul(tm3, ui3, dw_re8)
                        nc.vector.tensor_add(vi3, vi3, tm3)

                # pairs available after this half, with full back chain+gather
                for i in ([0] if h == 0 else [1, 2, 3, 4, 5]):
                    mi, mj = PAIRS[i]
                    vi_re, vi_im = mview(v_mre, mi), mview(v_mim, mi)
                    uj_re, uj_im = mview(u_mre, mj), mview(u_mim, mj)
                    q_re_i = p_q.tile([128, NBK, P2], FP, tag="q_re", name=f"q_re{i}")
                    q_im_i = p_q.tile([128, NBK, P2], FP, tag="q_im", name=f"q_im{i}")
                    tq = p_q.tile([128, NBK, P2], FP, tag="qtmp", name="qtmp")
                    nc.vector.tensor_mul(q_re_i[:], vi_re, uj_re)
                    nc.vector.tensor_mul(tq[:], vi_im, uj_im)
                    nc.vector.tensor_add(q_re_i[:], q_re_i[:], tq[:])
                    nc.vector.tensor_mul(q_im_i[:], vi_im, uj_re)
                    nc.vector.tensor_mul(tq[:], vi_re, uj_im)
                    nc.vector.tensor_sub(q_im_i[:], q_im_i[:], tq[:])
                    # transpose Q_i -> Qt + inverse inner, per bk-half
                    qf_re = q_re_i[:].rearrange("p bk k -> p (bk k)")
                    qf_im = q_im_i[:].rearrange("p bk k -> p (bk k)")
                    ire = psA.tile([128, 512], FP, tag="t6", name="ire")
                    iim = psA.tile([128, 512], FP, tag="t7", name="iim")
                    for b4 in range(2):
                        qt_re = p_bk.tile([64, 512], FP, tag="qt_re", name=f"qt_re{i}_{b4}")
                        qt_im = p_bk.tile([64, 512], FP, tag="qt_im", name=f"qt_im{i}_{b4}")
                        for srcf, dst in [(qf_re, qt_re), (qf_im, qt_im)]:
                            pt = psA.tile([64, 512], FP, tag="t3", name="ptq")
                            for k in range(4):
                                bkk = b4 * 4 + k
                                nc.tensor.transpose(
                                    pt[:, k * 128 : (k + 1) * 128],
                                    srcf[:, bkk * 64 : (bkk + 1) * 64], ct["ident"][:],
                                )
                            nc.scalar.copy(dst[:], pt[:])
                        for pi in range(4):
                            bkpi = b4 * 4 + pi
                            lre = qt_re[:, pi * 128 : (pi + 1) * 128]
                            lim = qt_im[:, pi * 128 : (pi + 1) * 128]
                            osl = slice(bkpi * 64, (bkpi + 1) * 64)
                            nc.tensor.matmul(ire[:, osl], lre, ct["e64_re"][:], start=True, stop=False)
                            nc.tensor.matmul(ire[:, osl], lim, ct["e64_im_neg"][:], start=False, stop=True)
                            nc.tensor.matmul(iim[:, osl], lre, ct["e64_im"][:], start=True, stop=False)
                            nc.tensor.matmul(iim[:, osl], lim, ct["e64_re"][:], start=False, stop=True)
                    in_re = p_bk.tile([128, 512], FP, tag="in_re", name=f"in_re{i}")
                    in_im = p_bk.tile([128, 512], FP, tag="in_im", name=f"in_im{i}")
                    t_a = p_bk.tile([128, 512], FP, tag="t_a", name="t_a")
                    ire3 = ire[:].rearrange("p (r t) -> p r t", t=P2)
                    iim3 = iim[:].rearrange("p (r t) -> p r t", t=P2)
                    ta3 = t_a[:].rearrange("p (r t) -> p r t", t=P2)
                    inre3 = in_re[:].rearrange("p (r t) -> p r t", t=P2)
                    inim3 = in_im[:].rearrange("p (r t) -> p r t", t=P2)
                    nc.vector.tensor_mul(ta3, ire3, twir_b)
                    nc.vector.tensor_mul(inre3, iim3, twii_b)
                    nc.vector.tensor_sub(inre3, ta3, inre3)
                    nc.vector.tensor_mul(ta3, ire3, twii_b)
                    nc.vector.tensor_mul(inim3, iim3, twir_b)
                    nc.vector.tensor_add(inim3, inim3, ta3)
                    # inverse outer -> tt_i [NT1, NBK*P2]
                    ot = psA.tile([NT1, 512], FP, tag="t4", name="ot")
                    nc.tensor.matmul(ot[:], ct["e128_re"][:], in_re[:], start=True, stop=False)
                    nc.tensor.matmul(ot[:], ct["e128_im_neg"][:], in_im[:], start=False, stop=True)
                    tt_i = p_bk.tile([NT1, 512], FP, tag="tt", name=f"tt{i}")
                    nc.scalar.copy(tt_i[:], ot[:])
                    # per-pair table: [NBK, NTAB] via 64 small transposes
                    tt3 = tt_i[:].rearrange("a (bk t) -> a bk t", bk=NBK)
                    ptab = psA.tile([NBK, NTAB], FP, tag="t5", name="ptab")
                    for t2v in range(P2):
                        nc.tensor.transpose(
                            ptab[:, t2v * NT1 : (t2v + 1) * NT1],
                            tt3[:, :, t2v : t2v + 1],
                            ct["ident"][0:NT1, 0:NT1],
                        )
                    tabs_i = p_bk.tile([NBK, NTAB], FP, tag="tabs", name=f"tabs{i}")
                    nc.scalar.copy(tabs_i[:], ptab[:])
                    # replicate each bk row across its 16 partitions
                    for halfc in range(2):
                        sl = slice(halfc * 512, (halfc + 1) * 512)
                        prep = psA.tile([128, 512], FP, tag="t6", name="prep")
                        nc.tensor.matmul(prep[:], ct["repl16"][:], tabs_i[:, sl])
                        nc.scalar.copy(gdata[i][:, sl], prep[:])
                    if i == 0:
                        # pair 0 gathers straight into the accumulator
                        for gh in range(2):
                            hsl = slice(gh * NQH, (gh + 1) * NQH)
                            if gh == 0:
                                nc.gpsimd.load_library(library_config.ap_gather)
                            nc.gpsimd.ap_gather(
                                acc[:, hsl], gdata[i][:],
                                gidx[i][:, gh * 512 : (gh + 1) * 512],
                                channels=128, num_elems=NTAB, d=1, num_idxs=NQH,
                            )
                    else:
                        for gh in range(2):
                            hsl = slice(gh * NQH, (gh + 1) * NQH)
                            gq = p_gb.tile([128, NQH], FP, tag="gout", name="gout")
                            nc.gpsimd.ap_gather(
                                gq[:], gdata[i][:],
                                gidx[i][:, gh * 512 : (gh + 1) * 512],
                                channels=128, num_elems=NTAB, d=1, num_idxs=NQH,
                            )
                            nc.vector.tensor_add(acc[:, hsl], acc[:, hsl], gq[:])
                            if i == 5 and gh == 0:
                                nc.vector.tensor_reduce(
                                    m128a[:], acc[:, 0:NQH],
                                    axis=mybir.AxisListType.X, op=ALU.max,
                                )

            p_bk.close()
            p_n.close()
            p_uv.close()
            p_yt.close()
            psA.close()
            p_z.close()
            p_a.close()
            p_ytw.close()
            p_gb.close()

            # ---------------- normalize + out ----------------
            with ExitStack() as gph:
                op = gph.enter_context(tc.tile_pool(name="gout2", bufs=1))
                psn = gph.enter_context(tc.tile_pool(name="psn", bufs=1, space="PSUM"))
                m128 = op.tile([128, 1], FP, tag="m128", name="m128")
                nc.vector.tensor_reduce(
                    m128[:], acc[:, NQH:], axis=mybir.AxisListType.X, op=ALU.max
                )
                nc.vector.tensor_tensor(m128[:], m128[:], m128a[:], op=ALU.max)
                mt = psn.tile([1, 128], FP, tag="mt", name="mt")
                nc.tensor.transpose(mt[:], m128[:], ct["ident"][:])
                mg = op.tile([1, NBK], FP, tag="mg", name="mg")
                nc.vector.tensor_reduce(
                    mg[:],
                    mt[:].rearrange("a (g r) -> a g r", r=16),
                    axis=mybir.AxisListType.X, op=ALU.max,
                )
                nc.vector.reciprocal(mg[:], mg[:])
                mgt = psn.tile([NBK, 1], FP, tag="mgt", name="mgt")
                nc.tensor.transpose(mgt[:], mg[:], ct["ident"][0:1, 0:1])
                mgs = op.tile([NBK, 1], FP, tag="mgs", name="mgs")
                nc.scalar.copy(mgs[:], mgt[:])
                scp = psn.tile([128, 1], FP, tag="scp", name="scp")
                nc.tensor.matmul(scp[:], ct["repl16"][:], mgs[:])
                sc = op.tile([128, 1], FP, tag="sc", name="sc")
                nc.scalar.copy(sc[:], scp[:])
                for ch in range(NQ // 4096):
                    sl = slice(ch * 4096, (ch + 1) * 4096)
                    if ch < 2:
                        grids = op.tile([128, 4096], FP, tag="grids_s", name="grids_s")
                        nc.scalar.activation(
                            grids[:], acc[:, sl], ACTF.Copy, scale=sc[:, 0:1]
                        )
                    else:
                        grids = op.tile([128, 4096], FP, tag="grids_v", name="grids_v")
                        nc.vector.tensor_scalar_mul(grids[:], acc[:, sl], sc[:, 0:1])
                    gv = grids[:].rearrange("(g r) q -> g r q", r=16)[:, 0, :]
                    nc.sync.dma_start(out_d[:, sl], gv)
            p_q.close()
            p_go.close()

    nc.compile()
    return nc


_NC_CACHE = {}


def kernel(signal, mic_coordinates, room_dims):
    signal = np.ascontiguousarray(np.asarray(signal, dtype=np.float32))
    mic_coordinates = np.ascontiguousarray(np.asarray(mic_coordinates, dtype=np.float32))
    room_dims = np.ascontiguousarray(np.asarray(room_dims, dtype=np.float32))
    if "nc" not in _NC_CACHE:
        _NC_CACHE["nc"] = build_kernel()
        _NC_CACHE["consts"] = make_consts()
    nc = _NC_CACHE["nc"]
    consts = _NC_CACHE["consts"]
    in_maps = []
    for core in range(8):
        m = shard_core(signal, mic_coordinates, room_dims, core)
        m.update(consts)
        in_maps.append(m)
    res = run_bass_kernel_spmd(nc, in_maps, core_ids=list(range(8)), trace=False)
    outs = [res.results[c]["out"].reshape(NB * K, NQ) for c in range(8)]
    return np.concatenate(outs, axis=0).reshape(B, K, NQ).astype(np.float32)

